# revision 16
# baseline (speedup 1.0000x reference)
"""DistMult edge scoring on Trainium2 (Bass/Tile), 8-core edge-parallel.

score[e] = sigmoid(sum_d h[src_e]*W[rel_e]*h[dst_e]) for 1.5M edges.

Sharding: edges are split evenly across the 8 NeuronCores (edge/data
parallel); h and W are replicated to every core.

Per-core strategy (v13):
  - Node table is viewed as 4 blocks of 25000 rows so row indices fit the
    int16 index format of the bulk DMA-gather instruction.  Edges are
    bucketed GLOBALLY into 16 (src_block, dst_block) groups; each group's
    edge list is split into 8 equal floor(N_g/8) per-core slices so every
    core's chunk fill counts are identical and can be baked into the
    program as num_idxs_reg constants (the <=7 leftover edges per group,
    <=112 of 1.5M total, are scored exactly on the host).  Each per-core
    group slice is padded with index -1 to a whole number of 2048-edge
    chunks: the gather ucode trims trailing negative indices, skipping
    their descriptor work (the ~4.6% padding tax), which matters because
    Q7 descriptor generation is the kernel's bottleneck.
  - h is shipped as bf16; per chunk two dma_gather ops (non-transpose --
    the transpose path's xbar spray corrupts when gathers overlap) pull
    u and v as [128, 24, 128] bf16 tiles (edge j -> partition j%128,
    slot j//128; 256 B/row).  The gathers round-robin over 4 SWDGE
    queues so their Q7 descriptor generation pipelines across all 4
    core pairs (queue q -> cores 2q,2q+1); a single queue serializes at
    ~16.4 us per 2048-row gather and dominates the whole kernel.
  - W[rel] is built on-chip from a host-precomputed bf16 one-hot
    [6, 3072] per chunk: one K=6 matmul per 128-edge slot against
    W (bf16) reconstructs W[rel] in PSUM as fp32.
  - DVE: m = u*v (bf16), p2 = m*hr (bf16 out), 3D reduce over the
    hidden dim -> [128, 16] fp32 scores.
  - ACT applies the final sigmoid once over the whole score buffer.
"""

import os
import sys

import numpy as np

# ---- problem constants (hardcoded; harness contract) ----
N_NODES = 100000
N_EDGES = 1500000
N_RELS = 6
D = 128
N_CORES = 8

_NBLK = 4
_BLK = -(-N_NODES // _NBLK)          # 25000 rows per block (< 32768: int16 ok)
_K = 24                              # 128-edge slots per chunk
_C = 128 * _K                        # 3072 edges per chunk
_NGRP = _NBLK * _NBLK                # 16 (src_block, dst_block) groups
_NQ = 4                              # SWDGE queues (Q7 core pairs)
_SLICE = 16                          # chunks per sigmoid/output slice


def _import_concourse():
    try:
        import concourse  # noqa: F401
    except ModuleNotFoundError:
        for p in ("/opt/trn_rl_repo", "/root/.axon_site/_ro/trn_rl_repo"):
            if os.path.isdir(p) and p not in sys.path:
                sys.path.insert(0, p)
        import concourse  # noqa: F401


def build_bass(n_nodes, blk, d, k, cap, last_counts, num_devices):
    """Build + compile the per-core Bass/Tile program.

    cap = chunks per (src_block, dst_block) group; n_chunks = 16 * cap.
    last_counts[g] = valid edges in group g's final chunk (same on every
    core by construction); earlier chunks are full (2048 valid).
    """
    _import_concourse()
    import concourse.bacc as bacc
    import concourse.tile as tile
    from concourse import mybir

    f32 = mybir.dt.float32
    bf16 = mybir.dt.bfloat16
    i16 = mybir.dt.int16
    mult = mybir.AluOpType.mult
    C = 128 * k
    n_chunks = _NGRP * cap

    nc = bacc.Bacc(
        "TRN2",
        target_bir_lowering=False,
        debug=False,
        enable_asserts=True,
        num_devices=num_devices,
        num_swdge_queues=_NQ,
    )
    h = nc.dram_tensor("h", [n_nodes, d], bf16, kind="ExternalInput").ap()
    w6 = nc.dram_tensor("w6", [N_RELS, d], bf16, kind="ExternalInput").ap()
    srcw = nc.dram_tensor("srcw", [n_chunks, 128, C // 16], i16,
                          kind="ExternalInput").ap()
    dstw = nc.dram_tensor("dstw", [n_chunks, 128, C // 16], i16,
                          kind="ExternalInput").ap()
    ohd = nc.dram_tensor("oh", [n_chunks, N_RELS, C], bf16,
                         kind="ExternalInput").ap()
    out = nc.dram_tensor("out", [128, n_chunks * k], f32,
                         kind="ExternalOutput").ap()

    with tile.TileContext(nc) as tc:
        with tc.tile_pool(name="const", bufs=1) as constp, \
             tc.tile_pool(name="idxp", bufs=8) as idxp, \
             tc.tile_pool(name="gat", bufs=6) as gat, \
             tc.tile_pool(name="ohp", bufs=4) as ohp, \
             tc.tile_pool(name="mp", bufs=3) as mp, \
             tc.tile_pool(name="psum", bufs=2, space="PSUM") as psum, \
             tc.tile_pool(name="outp", bufs=1) as outp:
            wtile = constp.tile([N_RELS, d], bf16)
            nc.sync.dma_start(out=wtile[:, :], in_=w6[:, :])
            score_buf = outp.tile([128, n_chunks * k], f32)
            sig_buf = outp.tile([128, n_chunks * k], f32)

            # dummy 128-index gather absorbs the one-time Q7 IRAM load for
            # the gather ucode while the first real index DMAs are in flight.
            warm_idx = constp.tile([128, 8], i16)
            nc.vector.memset(warm_idx[:, :], 0)
            warm_out = constp.tile([128, 1, d], bf16)
            nc.gpsimd.dma_gather(
                out_ap=warm_out[:, :, :],
                in_ap=h[0:blk, :],
                idxs_ap=warm_idx[:, :],
                num_idxs=128,
                num_idxs_reg=128,
                elem_size=d,
                single_packet=False,
                queue_num=0,
            )

            # process full chunks first, then all tail chunks back-to-back:
            # the 4-queue pipeline's cycle time anchors on the largest gather
            # in each concurrent group, so shortened tail gathers only pay
            # off when they run together.
            order = [c for c in range(n_chunks) if (c % cap) < cap - 1] + \
                    [c for c in range(n_chunks) if (c % cap) == cap - 1]
            for ci, c in enumerate(order):
                g = c // cap
                bi, bj = g // _NBLK, g % _NBLK
                qu = (2 * ci) % _NQ
                qv = (2 * ci + 1) % _NQ
                cnt = C if (c % cap) < cap - 1 else int(last_counts[g])
                kp = -(-cnt // 128)
                ni = kp * 128

                src_t = idxp.tile([128, C // 16], i16, tag="src")
                nc.sync.dma_start(out=src_t[:, :], in_=srcw[c])
                dst_t = idxp.tile([128, C // 16], i16, tag="dst")
                nc.sync.dma_start(out=dst_t[:, :], in_=dstw[c])
                oh_t = ohp.tile([N_RELS, C], bf16, tag="oh")
                nc.sync.dma_start(out=oh_t[:, :], in_=ohd[c])

                u_t = gat.tile([128, k, d], bf16, tag="u")
                nc.gpsimd.dma_gather(
                    out_ap=u_t[:, :kp, :],
                    in_ap=h[bi * blk:(bi + 1) * blk, :],
                    idxs_ap=src_t[:, :ni // 16],
                    num_idxs=ni,
                    num_idxs_reg=cnt,
                    elem_size=d,
                    single_packet=False,
                    queue_num=qu,
                )
                v_t = gat.tile([128, k, d], bf16, tag="v")
                nc.gpsimd.dma_gather(
                    out_ap=v_t[:, :kp, :],
                    in_ap=h[bj * blk:(bj + 1) * blk, :],
                    idxs_ap=dst_t[:, :ni // 16],
                    num_idxs=ni,
                    num_idxs_reg=cnt,
                    elem_size=d,
                    single_packet=False,
                    queue_num=qv,
                )

                # two 12-slot compute halves keep the PSUM W[rel] tiles
                # double-buffered (3 banks x 2 bufs); a full-chunk tile
                # would serialize the PE->DVE chain across chunks.
                for h0 in range(0, kp, k // 2):
                    hs = min(k // 2, kp - h0)
                    hr = psum.tile([128, k // 2, d], f32, tag="hr")
                    for kk in range(hs):
                        nc.tensor.matmul(
                            out=hr[:, kk, :],
                            lhsT=oh_t[:, (h0 + kk) * 128:(h0 + kk + 1) * 128],
                            rhs=wtile[:, :],
                            start=True,
                            stop=True,
                        )

                    m_t = mp.tile([128, k // 2, d], bf16, tag="m")
                    nc.vector.tensor_tensor(
                        out=m_t[:, :hs, :], in0=u_t[:, h0:h0 + hs, :],
                        in1=v_t[:, h0:h0 + hs, :],
                        op=mult,
                    )
                    p2_t = mp.tile([128, k // 2, d], bf16, tag="p2")
                    nc.vector.tensor_tensor(
                        out=p2_t[:, :hs, :], in0=m_t[:, :hs, :],
                        in1=hr[:, :hs, :],
                        op=mult,
                    )
                    nc.vector.tensor_reduce(
                        out=score_buf[:, ci * k + h0:ci * k + h0 + hs],
                        in_=p2_t[:, :hs, :],
                        axis=mybir.AxisListType.X,
                        op=mybir.AluOpType.add,
                    )

                # big sigmoid+output slice while the last chunks still
                # gather; only the final 2 chunks' scores drain in the tail.
                if ci == n_chunks - 3:
                    cut = (n_chunks - 2) * k
                    nc.scalar.activation(
                        out=sig_buf[:, :cut], in_=score_buf[:, :cut],
                        func=mybir.ActivationFunctionType.Sigmoid,
                    )
                    nc.sync.dma_start(out=out[:, :cut], in_=sig_buf[:, :cut])

            cut = (n_chunks - 2) * k
            nc.scalar.activation(
                out=sig_buf[:, cut:], in_=score_buf[:, cut:],
                func=mybir.ActivationFunctionType.Sigmoid,
            )
            nc.sync.dma_start(out=out[:, cut:], in_=sig_buf[:, cut:])
    nc.compile()
    return nc


_BUILT = {}


def _get_built(cap, last_counts):
    key = (N_NODES, _BLK, D, _K, cap, tuple(last_counts), N_CORES)
    if key not in _BUILT:
        _BUILT[key] = build_bass(N_NODES, _BLK, D, _K, cap, last_counts,
                                 N_CORES)
    return _BUILT[key]


def _wrap_idx(slot_arr, n_chunks, C):
    """[n_chunks*C] int16 -> [n_chunks, 128, C//16] wrapped+replicated."""
    a = slot_arr.reshape(n_chunks, C // 16, 16).transpose(0, 2, 1)  # [nc,16,C/16]
    return np.ascontiguousarray(np.tile(a, (1, 8, 1)))


def _plan(src, dst):
    """Global grouping: per (src_blk,dst_blk) group, equal floor(N_g/8)
    per-core slices (leftovers -> host).  Returns per-core position lists,
    leftover positions, cap and per-group last-chunk counts."""
    g = (src // _BLK) * _NBLK + (dst // _BLK)
    order = np.argsort(g, kind="stable")
    counts = np.bincount(g, minlength=_NGRP)
    per_core = counts // N_CORES
    cap = int(-(-per_core.max() // _C))
    last_counts = per_core - (cap - 1) * _C
    assert (last_counts > 0).all() and (last_counts <= _C).all()
    core_pos = [[] for _ in range(N_CORES)]
    leftover = []
    ofs = 0
    for gi in range(_NGRP):
        n, pc = int(counts[gi]), int(per_core[gi])
        blockpos = order[ofs:ofs + n]
        ofs += n
        for core in range(N_CORES):
            core_pos[core].append(blockpos[core * pc:(core + 1) * pc])
        leftover.append(blockpos[N_CORES * pc:])
    leftover = np.concatenate(leftover) if leftover else np.empty(0, np.int64)
    return core_pos, leftover, cap, last_counts


def _prep_core(src, dst, rel, group_pos, cap):
    """Fill one core's slot arrays from its per-group edge positions.

    Padding slots get index -1 (the gather ucode trims trailing
    negatives, skipping their descriptor work)."""
    import ml_dtypes
    n_chunks = _NGRP * cap
    nslot = n_chunks * _C
    src_slot = np.full(nslot, -1, np.int16)
    dst_slot = np.full(nslot, -1, np.int16)
    rel_slot = np.zeros(nslot, np.int64)
    perm = np.full(nslot, -1, np.int64)
    for gi in range(_NGRP):
        idxs = group_pos[gi]
        n = len(idxs)
        base = gi * cap * _C
        src_slot[base:base + n] = (src[idxs] - (gi // _NBLK) * _BLK).astype(np.int16)
        dst_slot[base:base + n] = (dst[idxs] - (gi % _NBLK) * _BLK).astype(np.int16)
        rel_slot[base:base + n] = rel[idxs]
        perm[base:base + n] = idxs
    srcw = _wrap_idx(src_slot, n_chunks, _C)
    dstw = _wrap_idx(dst_slot, n_chunks, _C)
    oh = (rel_slot.reshape(n_chunks, 1, _C)
          == np.arange(N_RELS).reshape(1, N_RELS, 1))
    oh = np.ascontiguousarray(oh.astype(ml_dtypes.bfloat16))
    return srcw, dstw, oh, perm


def _h_w6(h, W):
    import ml_dtypes
    h16 = np.ascontiguousarray(
        np.asarray(h, dtype=np.float32).astype(ml_dtypes.bfloat16))
    w6 = np.ascontiguousarray(
        np.asarray(W, dtype=np.float32).astype(ml_dtypes.bfloat16))
    return h16, w6


def _make_in_maps(h, W, src, dst, rel, core_pos, cap):
    h16, w6 = _h_w6(h, W)
    in_maps, perms = [], []
    for core in range(N_CORES):
        srcw, dstw, oh, perm = _prep_core(src, dst, rel, core_pos[core], cap)
        in_maps.append({
            "h": h16, "w6": w6, "srcw": srcw, "dstw": dstw, "oh": oh,
        })
        perms.append(perm)
    return in_maps, perms


def _host_scores(h, W, src, dst, rel, pos):
    """Exact fp32 scores for the <=112 leftover edges."""
    if len(pos) == 0:
        return np.empty(0, np.float32)
    h32 = np.asarray(h, dtype=np.float32)
    W32 = np.asarray(W, dtype=np.float32)
    s = np.sum(h32[src[pos]] * W32[rel[pos]] * h32[dst[pos]], axis=1)
    return (1.0 / (1.0 + np.exp(-s))).astype(np.float32)


def _unshard(results, perms, leftover, leftover_scores, cap):
    n_chunks = _NGRP * cap
    # device score columns are in processing order (full chunks first,
    # then tails); invert that permutation back to chunk order.
    order = [c for c in range(n_chunks) if (c % cap) < cap - 1] + \
            [c for c in range(n_chunks) if (c % cap) == cap - 1]
    inv = np.argsort(np.asarray(order))
    full = np.empty(N_EDGES, np.float32)
    for core in range(N_CORES):
        o = np.asarray(results[core]["out"])  # [128, n_chunks*K]
        flat = o.reshape(128, n_chunks, _K)[:, inv, :].transpose(1, 2, 0).reshape(-1)
        perm = perms[core]
        mask = perm >= 0
        full[perm[mask]] = flat[mask]
    full[leftover] = leftover_scores
    return full


def _axon_reset():
    try:
        import ctypes
        lib = ctypes.CDLL("/opt/axon/libaxon_pjrt.so")
        if hasattr(lib, "axon_reset"):
            lib.axon_reset()
    except Exception:
        pass


def _run(nc, in_maps, trace=False, trace_kwargs=None):
    from concourse.bass_utils import run_bass_kernel_spmd

    # A previous process can leave the accelerator wedged
    # (NRT_EXEC_UNIT_UNRECOVERABLE); reset and retry up to twice.
    for attempt in range(3):
        try:
            return run_bass_kernel_spmd(
                nc,
                in_maps,
                core_ids=list(range(N_CORES)),
                trace=trace,
                **(trace_kwargs or {}),
            )
        except Exception:
            if attempt == 2:
                raise
            _axon_reset()


def _prepare(h, W, src, dst, rel):
    src32 = np.asarray(src, dtype=np.int32)
    dst32 = np.asarray(dst, dtype=np.int32)
    rel32 = np.asarray(rel, dtype=np.int32)
    core_pos, leftover, cap, last_counts = _plan(src32, dst32)
    nc = _get_built(cap, last_counts)
    in_maps, perms = _make_in_maps(h, W, src32, dst32, rel32, core_pos, cap)
    lscore = _host_scores(h, W, src32, dst32, rel32, leftover)
    return nc, in_maps, perms, leftover, lscore, cap


def kernel(h, W, src, dst, rel):
    nc, in_maps, perms, leftover, lscore, cap = _prepare(h, W, src, dst, rel)
    res = _run(nc, in_maps)
    return _unshard(res.results, perms, leftover, lscore, cap)


# used by test.py for profiling runs
def kernel_traced(h, W, src, dst, rel, **trace_kwargs):
    nc, in_maps, perms, leftover, lscore, cap = _prepare(h, W, src, dst, rel)
    res = _run(nc, in_maps, trace=True, trace_kwargs=trace_kwargs)
    return _unshard(res.results, perms, leftover, lscore, cap), res
